# revision 20
# baseline (speedup 1.0000x reference)
"""ARIMA(4,1,2)+exog Trainium2 kernel, data-parallel over 8 NeuronCores.

Per batch row (derived from the reference):
  m=4; steps = T-1-m
  e_i = sum_{j=0..5} g_j x[i+j] - feat_i - bias       (feat_i = features[i+4] . w)
  res'_i = e_i - c1 res'_{i-1} - c0 res'_{i-2}  (zero IC; c0,c1 = ma_coef)
  out[0] = x[0]; out[i+1] = x0 - x4 + x[i+5] - cumsum(res')_i - c1 e0 V_i
The IIR 1/A(z) is an exact-to-f32 FIR via root-doubling (truncated where the
taps drop below f32 noise):
  v1 = e + d1 e(-4);  v2 = v1 - beta v1(-2) + gam v1(-4);
  res = v2 - c1 v2(-1) + c0 v2(-2)

Device layout (per core, 32 rows): partitions p = 32*q + r fold each row's
timeline into NQ=4 quarters of TQ=4096 (host pre-folds x and features and
un-folds the output). Features are bf16 (tolerance budget) so the 512MB
stream halves; the F-reduction runs on TensorE with block-diagonal weights,
streamed in column chunks that pipeline with the DVE work. The last chunks
shrink so the unavoidable post-stream DVE chain is short. The cross-quarter
FIR boundary and quarter cumsum offsets are deferred linear corrections.
All large DMAs use >=512-row access patterns so descriptors spread across
all 16 DMA queues (32-row blocks map to queues).
"""

import numpy as np

import concourse.bass as bass
import concourse.bacc as bacc
import concourse.mybir as mybir
import concourse.tile as tile
from concourse.bass_utils import run_bass_kernel_spmd

FP = mybir.dt.float32
BF = mybir.dt.bfloat16
OP = mybir.AluOpType

B, T, F = 256, 16384, 32
NCORES = 8
R = B // NCORES            # 32 rows per core
M_LAG = 4
STEPS = T - 1 - M_LAG      # 16379

NQ = 4                     # fold factor: partition p = 32*q + r
TQ = T // NQ               # 4096
SIZES = [1024, 1024, 1024, 896, 128]   # chunk widths, sum == TQ
MMN = 512                  # max matmul free dim (one PSUM bank)
PATCH = 32                 # quarter-head patch width (> FIR span 14)
XW = TQ + 8                # folded-x row width
VW = 64                    # columns of explicit V correction (V converges)

LAST_RESULT = None


def _fir_taps(c0, c1):
    beta = 2.0 * c0 - c1 * c1
    gam = c0 * c0
    p = 2.0 * gam - beta * beta
    return beta, gam, -p          # d1 = -p


def build_nc(ar, c0, c1, bias, vinf):
    g = [0.0] * 6
    g[5] += 1.0
    g[4] -= 1.0
    for k in range(4):
        g[k] += ar[k]
        g[k + 1] -= ar[k]
    beta, gam, d1 = _fir_taps(c0, c1)
    sizes = SIZES
    assert sum(sizes) == TQ
    ncht = len(sizes)
    chmax = max(sizes)

    nc = bacc.Bacc(None, target_bir_lowering=False)
    xp_d = nc.declare_dram_parameter("xp", [128, XW], FP, isOutput=False)
    ft_d = nc.declare_dram_parameter("ft", [8, 128, NQ, TQ], BF, isOutput=False)
    w_d = nc.declare_dram_parameter("wmat", [128, 8 * 32], BF, isOutput=False)
    v_d = nc.declare_dram_parameter("vsmall", [R, VW], FP, isOutput=False)
    qm_d = nc.declare_dram_parameter("qmask", [128, 128], FP, isOutput=False)
    out_d = nc.declare_dram_parameter("out", [128, TQ], BF, isOutput=True)

    def stt(out, in0, scl, in1, eng=None):
        (eng or nc.vector).scalar_tensor_tensor(
            out, in0, float(scl), in1, OP.mult, OP.add
        )

    with tile.TileContext(nc) as tc:
        with (
            tc.tile_pool(name="fixed", bufs=1) as fixed,
            tc.tile_pool(name="gtiles", bufs=6) as gpool,
            tc.tile_pool(name="fpool", bufs=2) as fpool,
            tc.tile_pool(name="scr", bufs=1) as scr,
            tc.tile_pool(name="rpool", bufs=2) as rpool,
            tc.tile_pool(name="spool", bufs=2) as spool,
            tc.tile_pool(name="outp", bufs=len(sizes)) as outp,
            tc.tile_pool(name="small", bufs=1) as small,
            tc.tile_pool(name="psum", bufs=3, space=bass.MemorySpace.PSUM) as psum,
            tc.tile_pool(name="psoff", bufs=1, space=bass.MemorySpace.PSUM) as psoff,
        ):
            x_ext = fixed.tile([128, XW], FP)
            e_b = fixed.tile([128, TQ], FP)
            xband = fixed.tile([128, TQ], FP)
            wsb = fixed.tile([128, 8 * 32], BF)
            qmask = fixed.tile([128, 128], FP)
            vsm = fixed.tile([R, VW], FP)
            va = fixed.tile([128, PATCH + chmax], FP)
            vb = fixed.tile([128, PATCH + chmax], FP)
            vc = fixed.tile([128, PATCH + chmax], FP)

            # weights first on the sync queue: first matmuls need them
            nc.sync.dma_start(wsb[:], w_d[:, :])
            # aux loads on the gpsimd queue; 512-row views spread queues
            nc.gpsimd.dma_start(
                x_ext[:].rearrange("p (a b) -> p a b", a=4),
                bass.AP(xp_d, 0, [[XW, 128], [XW // 4, 4], [1, XW // 4]]),
            )
            nc.gpsimd.dma_start(vsm[:], v_d[:, :])
            nc.gpsimd.dma_start(qmask[:], qm_d[:, :])

            ones = small.tile([128, chmax], FP)
            nc.vector.memset(ones[:], 1.0)

            e0_bc = small.tile([128, 1], FP)
            cpp = small.tile([128, 1], FP)
            ccomb = small.tile([128, 1], FP)
            adj = small.tile([128, 1], FP)
            adj2 = small.tile([128, 1], FP)
            off_sb = small.tile([128, 1], FP)
            qsum2 = small.tile([128, 1], FP)
            res0h = small.tile([128, PATCH], FP)

            # cpp = x0 - x4 per row, broadcast to all quarters
            nc.vector.tensor_tensor(
                cpp[0:R, :], x_ext[0:R, 0:1], x_ext[0:R, 4:5], OP.subtract
            )
            for q in range(1, NQ):
                nc.gpsimd.dma_start(cpp[R * q:R * (q + 1), :], cpp[0:R, :])

            # ---- xband = sum_j g_j x(+j) - bias, precomputed up front ----
            PW = min(2048, TQ)
            for p0 in range(0, TQ, PW):
                pa_ = scr.tile([128, PW], FP, tag="xba")
                pb_ = scr.tile([128, PW], FP, tag="xbb")
                nc.vector.tensor_scalar(
                    pa_[:], x_ext[:, p0:p0 + PW],
                    float(g[0]), float(-bias), OP.mult, OP.add,
                )
                src, dst = pa_, pb_
                for j in range(1, 5):
                    stt(dst[:], x_ext[:, p0 + j:p0 + j + PW], g[j], src[:])
                    src, dst = dst, src
                stt(
                    xband[:, p0:p0 + PW],
                    x_ext[:, p0 + 5:p0 + 5 + PW], g[5], src[:],
                )

            s_tiles = [None] * ncht
            ot_tiles = [None] * ncht

            # ---------------- streamed main loop ----------------
            c0i = 0
            for c, sz in enumerate(sizes):
                pt = psum.tile([128, sz], FP, tag="pt")
                for gi in range(8):
                    gt = gpool.tile([128, NQ, sz], BF, tag="gt")
                    nc.sync.dma_start(
                        gt[:],
                        bass.AP(
                            ft_d,
                            gi * 128 * NQ * TQ + c0i,
                            [[NQ * TQ, 128], [TQ, NQ], [1, sz]],
                        ),
                    )
                    for q in range(NQ):
                        for h0 in range(0, sz, MMN):
                            hn = min(MMN, sz - h0)
                            nc.tensor.matmul(
                                pt[R * q:R * (q + 1), h0:h0 + hn],
                                wsb[:, 32 * gi:32 * (gi + 1)],
                                gt[:, q, h0:h0 + hn],
                                start=(gi == 0),
                                stop=(gi == 7),
                                tile_position=(0, R * q),
                                skip_group_check=True,
                            )
                ft_sb = fpool.tile([128, sz], FP, tag="ft_sb")
                nc.scalar.copy(ft_sb[:], pt[:])

                # ---- e = xband - feat (single op) ----
                stt(e_b[:, c0i:c0i + sz], ft_sb[:], -1.0,
                    xband[:, c0i:c0i + sz])
                if c == 0:
                    for q in range(NQ):
                        nc.gpsimd.dma_start(
                            e0_bc[R * q:R * (q + 1), :], e_b[0:R, 0:1]
                        )
                    nc.vector.scalar_tensor_tensor(
                        ccomb[:], e0_bc[:], float(vinf), cpp[:],
                        OP.mult, OP.add,
                    )

                # ---- FIR: 5 shifted multiply-adds ----
                lo2 = max(0, c0i - PATCH)
                ex2 = c0i + sz - lo2
                if c == 0:
                    # zero-IC edge handling for the first chunk
                    stt(va[:, 4:ex2], e_b[:, 0:ex2 - 4], d1, e_b[:, 4:ex2])
                    nc.vector.tensor_copy(va[:, 0:4], e_b[:, 0:4])
                else:
                    stt(va[:, 0:ex2], e_b[:, lo2 - 4:lo2 - 4 + ex2], d1,
                        e_b[:, lo2:lo2 + ex2])
                v1 = va
                stt(vb[:, 2:ex2], v1[:, 0:ex2 - 2], -beta, v1[:, 2:ex2])
                if c == 0:
                    nc.vector.tensor_copy(vb[:, 0:2], v1[:, 0:2])
                stt(vc[:, 4:ex2], v1[:, 0:ex2 - 4], gam, vb[:, 4:ex2])
                if c == 0:
                    nc.vector.tensor_copy(vc[:, 0:4], vb[:, 0:4])
                v2 = vc
                stt(va[:, 1:ex2], v2[:, 0:ex2 - 1], -c1, v2[:, 1:ex2])
                if c == 0:
                    nc.vector.tensor_copy(va[:, 0:1], v2[:, 0:1])
                r1 = va
                rt = rpool.tile([128, sz], FP, tag="rt")
                if c == 0:
                    stt(rt[:, 2:sz], v2[:, 0:sz - 2], c0, r1[:, 2:sz])
                    nc.vector.tensor_copy(rt[:, 0:2], r1[:, 0:2])
                    nc.vector.tensor_copy(res0h[:], rt[:, 0:PATCH])
                else:
                    stt(
                        rt[:], v2[:, ex2 - sz - 2:ex2 - 2],
                        c0, r1[:, ex2 - sz:ex2],
                    )

                # ---- cumsum chunk (unpatched; linear fixes deferred) ----
                st_ = spool.tile([128, sz], FP, tag="st")
                init = 0.0 if c == 0 else s_tiles[c - 1][:, sizes[c - 1] - 1:sizes[c - 1]]
                nc.vector.tensor_tensor_scan(
                    st_[:], ones[:, 0:sz], rt[:], init, OP.mult, OP.add
                )
                s_tiles[c] = st_

                # ---- output assembly: oA = x(i+5) - s ----
                otf = outp.tile([128, sz], FP, tag="otf")
                stt(otf[:], st_[:], -1.0, x_ext[:, c0i + 5:c0i + 5 + sz])
                if c == 0:
                    vtmp = small.tile([R, VW], FP)
                    nc.vector.scalar_tensor_tensor(
                        vtmp[:], vsm[:], e0_bc[0:R, :], otf[0:R, 0:VW],
                        OP.mult, OP.add,
                    )
                    nc.vector.tensor_copy(otf[0:R, 0:VW], vtmp[:])
                ot_tiles[c] = otf
                c0i += sz

            # ---------------- quarter-head patch as linear fix ----------
            W2 = 2 * PATCH
            pb = small.tile([128, W2], FP)
            pa = small.tile([128, W2], FP)
            pc = small.tile([128, W2], FP)
            pdd = small.tile([128, W2], FP)
            nc.vector.memset(pb[0:R, 0:PATCH], 0.0)
            nc.gpsimd.dma_start(pb[R:128, 0:PATCH], e_b[0:128 - R, TQ - PATCH:TQ])
            nc.vector.tensor_copy(pb[:, PATCH:W2], e_b[:, 0:PATCH])
            stt(pa[:, 4:W2], pb[:, 0:W2 - 4], d1, pb[:, 4:W2])
            nc.vector.tensor_copy(pa[:, 0:4], pb[:, 0:4])
            v1p = pa
            stt(pc[:, 2:W2], v1p[:, 0:W2 - 2], -beta, v1p[:, 2:W2])
            nc.vector.tensor_copy(pc[:, 0:2], v1p[:, 0:2])
            stt(pdd[:, 4:W2], v1p[:, 0:W2 - 4], gam, pc[:, 4:W2])
            nc.vector.tensor_copy(pdd[:, 0:4], pc[:, 0:4])
            v2p = pdd
            r1p = pa
            stt(r1p[:, 1:W2], v2p[:, 0:W2 - 1], -c1, v2p[:, 1:W2])
            prs = small.tile([128, PATCH], FP)
            stt(prs[:], v2p[:, PATCH - 2:W2 - 2], c0, r1p[:, PATCH:W2])

            # delta = patched - unpatched on [0, PATCH); sD = cumsum(delta)
            dlt = small.tile([128, PATCH], FP)
            sdl = small.tile([128, PATCH], FP)
            nc.vector.tensor_tensor(dlt[:], prs[:], res0h[:], OP.subtract)
            nc.vector.tensor_tensor_scan(
                sdl[:], ones[:, 0:PATCH], dlt[:], 0.0, OP.mult, OP.add
            )
            sD_last = sdl[:, PATCH - 1:PATCH]

            # offsets: qsum = s_last + sD_last; off = qmask^T @ qsum
            nc.vector.tensor_tensor(
                qsum2[:], s_tiles[-1][:, sizes[-1] - 1:sizes[-1]], sD_last, OP.add
            )
            po = psoff.tile([128, 1], FP)
            nc.tensor.matmul(po[:], qmask[:], qsum2[:], start=True, stop=True)
            nc.scalar.copy(off_sb[:], po[:])
            # subtract (off + sD_last - ccomb) from every out column
            nc.vector.tensor_tensor(adj[:], off_sb[:], sD_last, OP.add)
            nc.vector.tensor_tensor(adj2[:], adj[:], ccomb[:], OP.subtract)

            # chunk-0 cols [0, PATCH) additionally need (sdl - sD_last)
            sfix = small.tile([128, PATCH], FP)
            nc.vector.tensor_scalar(
                sfix[:], sdl[:], sD_last, None, OP.subtract
            )
            nc.vector.tensor_tensor(
                ot_tiles[0][:, 0:PATCH], ot_tiles[0][:, 0:PATCH], sfix[:],
                OP.subtract,
            )

            c0i = 0
            for c, sz in enumerate(sizes):
                otf = ot_tiles[c]
                obf = outp.tile([128, sz], BF, tag="obf")
                nc.vector.tensor_scalar(
                    obf[:], otf[:], adj2[:], None, OP.subtract
                )
                nc.sync.dma_start(
                    bass.AP(
                        out_d, c0i, [[TQ, 128], [sz // 4, 4], [1, sz // 4]]
                    ),
                    obf[:].rearrange("p (a b) -> p a b", a=4),
                )
                c0i += sz

    nc.compile()
    return nc


def _host_prep(ma_coef, feature_weights):
    import ml_dtypes

    c0, c1 = float(ma_coef[0]), float(ma_coef[1])
    w = np.asarray(feature_weights, np.float64)

    v = np.zeros(T, np.float64)
    if STEPS > 1:
        v[1] = 1.0
        for j in range(2, STEPS):
            v[j] = -c1 * v[j - 1] - c0 * v[j - 2]
    V = np.cumsum(v)
    vinf = float(-c1 * V[TQ - 1])
    vs = (-c1 * V[:VW] - vinf).astype(np.float32)
    vsmall = np.ascontiguousarray(np.broadcast_to(vs, (R, VW)))

    # wsb[p, 32g+m] = w[4g + p%4] * delta(p//4, m), contiguous for fast DMA
    wmat = np.zeros((128, 8, 32), ml_dtypes.bfloat16)
    for gi in range(8):
        for r in range(32):
            for fp in range(4):
                wmat[4 * r + fp, gi, r] = w[4 * gi + fp]
    wmat = wmat.reshape(128, 8 * 32)

    qmask = np.zeros((128, 128), np.float32)
    for pq in range(NQ):
        for mq in range(NQ):
            if pq < mq:
                for r in range(R):
                    qmask[R * pq + r, R * mq + r] = 1.0
    return c0, c1, vinf, vsmall, wmat, qmask


def _fold_x(x_rows):
    """(R, T) -> folded (128, XW): xf[32q+r, j] = x[r, TQ*q+j] (0-padded)."""
    xpad = np.zeros((R, T + 16), np.float32)
    xpad[:, :T] = x_rows
    xf = np.empty((128, XW), np.float32)
    for q in range(NQ):
        xf[R * q:R * (q + 1)] = xpad[:, TQ * q:TQ * q + XW]
    return xf


def _fold_features(f_rows):
    """(R, T, F) -> bf16 (8, 128, NQ, TQ) with the +M_LAG shift baked in."""
    import ml_dtypes
    tmp = np.zeros((R, F, T), ml_dtypes.bfloat16)
    tmp[:, :, : T - M_LAG] = f_rows[:, M_LAG:, :].transpose(0, 2, 1)
    return np.ascontiguousarray(
        tmp.reshape(R, 8, 4, NQ, TQ).transpose(1, 0, 2, 3, 4)
    ).reshape(8, 128, NQ, TQ)


def _unfold_out(param, x_rows):
    """(128, TQ) device output -> (R, STEPS+1) final rows."""
    param = np.asarray(param, np.float32)
    full = param.reshape(NQ, R, TQ).transpose(1, 0, 2).reshape(R, T)
    out = np.empty((R, STEPS + 1), np.float32)
    out[:, 0] = x_rows[:, 0]
    out[:, 1:] = full[:, :STEPS]
    return out


def kernel(x, features, ar_coef, ma_coef, feature_weights, bias):
    global LAST_RESULT
    x = np.ascontiguousarray(np.asarray(x, np.float32))
    features = np.ascontiguousarray(np.asarray(features, np.float32))
    ar = [float(a) for a in np.asarray(ar_coef)]
    bi = float(np.asarray(bias).reshape(-1)[0])
    c0, c1, vinf, vsmall, wmat, qmask = _host_prep(ma_coef, feature_weights)

    nc = build_nc(ar, c0, c1, bi, vinf)

    in_maps = []
    for ci in range(NCORES):
        rs = slice(ci * R, (ci + 1) * R)
        in_maps.append({
            "xp": _fold_x(x[rs]),
            "ft": _fold_features(features[rs]),
            "wmat": wmat,
            "vsmall": vsmall,
            "qmask": qmask,
        })

    r = run_bass_kernel_spmd(nc, in_maps, core_ids=list(range(NCORES)))
    LAST_RESULT = r
    outs = [
        _unfold_out(np.asarray(r.results[ci]["out"]), x[ci * R:(ci + 1) * R])
        for ci in range(NCORES)
    ]
    return np.concatenate(outs, axis=0).astype(np.float32)
